# revision 13
# baseline (speedup 1.0000x reference)
"""GAT-style attention score kernel for 8 TRN2 NeuronCores, v8.

Computes out[i,j] = LeakyReLU(Wh[i]@a1 + Wh[j]@a2, slope=0.2) for
N=8192, D=64 -> [8192, 8192] f32 output.

Sharding: output rows across 8 cores ([1024, 8192] slab each).

Core idea (HW-verified): ACT applies its per-partition bias BEFORE the
activation table and the Prelu table honors the alpha operand, so with
s2 pre-broadcast (host sends s2b = tile(s2,128) f16) and s1 as an f32
per-partition column, one fused scalar op computes a whole block:

    out[p, f] = Prelu(s2b[p, f] + s1c[p])      # evac+bias+leaky, 1x

Vector covers the rest of each 128-row tile with ts_add (4x, f32
scalar AP) + ts_mul (4x) + tt_max (2x), all f16 SBUF.  No TensorE, no
PSUM.  Steady tiles: S = 2 Prelus on [0:4608] (~4.2us), V = 1 triple
on [4608:8192] (~4.0us).  The last tile shifts work toward V (which
finishes its stream earlier) to shorten the tail.

Output stream: f16 pieces on TWO queues - sync HWDGE carries the
scalar-half pieces, the otherwise-idle gpsimd SWDGE queue carries the
vector-half pieces (all queues share the 16 SDMA engines, which
round-robin at packet granularity).  Host upcasts f16 -> f32.

Startup: the 2MB s2b load is the ramp constraint.  Critical chunks
(s1c, s2b[0:1024], s2b[4608:6400]) ride the scalar HWDGE queue first;
the bulk rides gpsimd gated on s1c landing.  Every input DMA has a
DEDICATED semaphore (a shared counter can hit its threshold via mixed
per-engine completions of different DMAs - this corrupted one core in
two earlier versions).  Tile-0 work is split finer for a fast ramp.
"""

from contextlib import ExitStack

import numpy as np
import concourse.bass as bass
import concourse.mybir as mybir
from concourse.bass_utils import run_bass_kernel_spmd

N = 8192          # nodes
D = 64            # feature dim
M = 8             # cores
ROWS = N // M     # 1024 output rows per core
NT = ROWS // 128  # 8 row tiles of 128 partitions
SW = 4608         # scalar's columns [0:SW] (steady tiles)
SH = SW // 2      # 2304
VMID = SW + (N - SW) // 2  # 6400
TW = 3584         # last-tile scalar/vector split
TH = TW // 2      # 1792
TVM = TW + (N - TW) // 2   # 5888
NEG_SLOPE = 0.2
NOB = 6           # output tile ring depth
XBASE = TW        # x/m buffers cover cols [TW:N]

S0_OPS = [(0, 1024), (1024, SH), (SH, SW)]
V0_OPS = [(SW, VMID), (VMID, N)]

_cache = {}


def _so_val(t, j):
    # S ops: t0 has 3, others 2
    return (j + 1) if t == 0 else 3 + 2 * (t - 1) + j + 1


def _vo_val(t, j=0):
    # V ops: t0 has 2, t1..t6 have 1, t7 has 2
    if t == 0:
        return j + 1
    return 2 + (t - 1) + j + 1


def _build():
    nc = bass.Bass()
    f16 = mybir.dt.float16
    f32 = mybir.dt.float32

    s1c_ext = nc.declare_dram_parameter("s1c", [128, NT], f32, isOutput=False)
    s2b_ext = nc.declare_dram_parameter("s2b", [128, N], f16, isOutput=False)
    out_ext = nc.declare_dram_parameter("out", [ROWS, N], f16, isOutput=True)

    with ExitStack() as ctx:
        sb_s1c = ctx.enter_context(nc.sbuf_tensor("sb_s1c", [128, NT], f32))
        sb_s2b = ctx.enter_context(nc.sbuf_tensor("sb_s2b", [128, N], f16))
        sb_x = ctx.enter_context(nc.sbuf_tensor("sb_x", [128, N - XBASE], f16))
        sb_m = ctx.enter_context(nc.sbuf_tensor("sb_m", [128, N - XBASE], f16))
        sb_o = [
            ctx.enter_context(nc.sbuf_tensor(f"sb_o{i}", [128, N], f16))
            for i in range(NOB)
        ]
        sb_junk = ctx.enter_context(nc.sbuf_tensor("sb_junk", [128, 1], f32))
        dS1 = ctx.enter_context(nc.semaphore("dS1"))      # s1c
        dQ0 = ctx.enter_context(nc.semaphore("dQ0"))      # s2b[0:1024]
        dinB = ctx.enter_context(nc.semaphore("dinB"))    # s2b[4608:6400]
        dinA1 = ctx.enter_context(nc.semaphore("dinA1"))  # s2b[1024:2304]
        dinA2 = ctx.enter_context(nc.semaphore("dinA2"))  # s2b[2304:4608]
        dinC = ctx.enter_context(nc.semaphore("dinC"))    # s2b[6400:8192]
        so = ctx.enter_context(nc.semaphore("so"))
        vo = ctx.enter_context(nc.semaphore("vo"))
        dt = [ctx.enter_context(nc.semaphore(f"dt{t}")) for t in range(NT)]
        block = ctx.enter_context(nc.Block())

        def dtt(t):
            if t == 0 or t == NT - 1:
                return 80 if t == 0 else 64
            return 48

        @block.sync
        def _(sync):
            # scalar-half output pieces
            for t in range(NT):
                ob = sb_o[t % NOB]
                dst = out_ext[t * 128:(t + 1) * 128, :]
                if t == 0:
                    bounds = [(0, 1024), (1024, SH), (SH, SW)]
                elif t == NT - 1:
                    bounds = [(0, TH), (TH, TW)]
                else:
                    bounds = [(0, SH), (SH, SW)]
                for j, (lo, hi) in enumerate(bounds):
                    sync.wait_ge(so, _so_val(t, j))
                    sync.dma_start(dst[:, lo:hi], ob[:, lo:hi]).then_inc(dt[t], 16)

        @block.gpsimd
        def _(gpsimd):
            # bulk s2b load, deferred behind s1c so it can't starve the
            # scalar queue's critical chunks
            gpsimd.wait_ge(dS1, 16)
            gpsimd.dma_start(
                sb_s2b[:, 1024:SH], s2b_ext[:, 1024:SH]
            ).then_inc(dinA1, 16)
            gpsimd.dma_start(
                sb_s2b[:, SH:SW], s2b_ext[:, SH:SW]
            ).then_inc(dinA2, 16)
            gpsimd.dma_start(
                sb_s2b[:, VMID:N], s2b_ext[:, VMID:N]
            ).then_inc(dinC, 16)
            # vector-half output pieces on the SWDGE queue
            for t in range(NT):
                ob = sb_o[t % NOB]
                dst = out_ext[t * 128:(t + 1) * 128, :]
                if t == 0:
                    bounds = [(SW, VMID), (VMID, N)]
                elif t == NT - 1:
                    bounds = [(TW, TVM), (TVM, N)]
                else:
                    bounds = [(SW, N)]
                for j, (lo, hi) in enumerate(bounds):
                    gpsimd.wait_ge(vo, _vo_val(t, j))
                    gpsimd.dma_start(dst[:, lo:hi], ob[:, lo:hi]).then_inc(dt[t], 16)

        @block.scalar
        def _(scalar):
            scalar.dma_start(sb_s1c[:, :], s1c_ext[:, :]).then_inc(dS1, 16)
            scalar.dma_start(
                sb_s2b[:, 0:1024], s2b_ext[:, 0:1024]
            ).then_inc(dQ0, 16)
            scalar.dma_start(
                sb_s2b[:, SW:VMID], s2b_ext[:, SW:VMID]
            ).then_inc(dinB, 16)
            # warm the Prelu table while they fly
            scalar.activation(
                sb_junk[:, :], sb_junk[:, :],
                mybir.ActivationFunctionType.Prelu,
                bias=0.0, scale=1.0, alpha=NEG_SLOPE,
            )
            for t in range(NT):
                ob = sb_o[t % NOB]
                b = sb_s1c[:, t:t + 1]
                if t >= NOB:
                    scalar.wait_ge(dt[t - NOB], dtt(t - NOB))
                if t == 0:
                    ops = S0_OPS
                elif t == NT - 1:
                    ops = [(0, TH), (TH, TW)]
                else:
                    ops = [(0, SH), (SH, SW)]
                for j, (lo, hi) in enumerate(ops):
                    if t == 0:
                        if j == 0:
                            scalar.wait_ge(dS1, 16)
                            scalar.wait_ge(dQ0, 16)
                        elif j == 1:
                            scalar.wait_ge(dinA1, 16)
                        else:
                            scalar.wait_ge(dinA2, 16)
                    scalar.activation(
                        ob[:, lo:hi], sb_s2b[:, lo:hi],
                        mybir.ActivationFunctionType.Prelu,
                        bias=b, scale=1.0, alpha=NEG_SLOPE,
                    ).then_inc(so)

        @block.vector
        def _(vector):
            for t in range(NT):
                ob = sb_o[t % NOB]
                b = sb_s1c[:, t:t + 1]
                if t >= NOB:
                    vector.wait_ge(dt[t - NOB], dtt(t - NOB))
                if t == 0:
                    ops = V0_OPS
                elif t == NT - 1:
                    ops = [(TW, TVM), (TVM, N)]
                else:
                    ops = [(SW, N)]
                for j, (lo, hi) in enumerate(ops):
                    if t == 0:
                        if j == 0:
                            vector.wait_ge(dS1, 16)
                            vector.wait_ge(dinB, 16)
                        else:
                            vector.wait_ge(dinC, 16)
                    x = sb_x[:, lo - XBASE:hi - XBASE]
                    m = sb_m[:, lo - XBASE:hi - XBASE]
                    vector.tensor_scalar_add(x, sb_s2b[:, lo:hi], b)
                    vector.tensor_scalar_mul(m, x, NEG_SLOPE)
                    vector.tensor_max(ob[:, lo:hi], x, m).then_inc(vo)

    return nc


def _run(Wh, a, trace=False, **kw):
    Wh = np.ascontiguousarray(np.asarray(Wh, dtype=np.float32))
    a = np.ascontiguousarray(np.asarray(a, dtype=np.float32))
    assert Wh.shape == (N, D) and a.shape == (2 * D, 1)

    if "nc" not in _cache:
        _cache["nc"] = _build()
    nc = _cache["nc"]

    s1 = Wh @ a[:D, 0]                         # [N] f32 row contribution
    s2b = np.ascontiguousarray(
        np.broadcast_to((Wh @ a[D:, 0]).astype(np.float16), (128, N))
    )

    in_maps = []
    for i in range(M):
        sl = s1[i * ROWS:(i + 1) * ROWS]
        s1c = np.ascontiguousarray(sl.reshape(NT, 128).T.astype(np.float32))
        in_maps.append({"s1c": s1c, "s2b": s2b})

    res = run_bass_kernel_spmd(nc, in_maps, core_ids=list(range(M)), trace=trace, **kw)
    out = np.concatenate(
        [res.results[i]["out"].astype(np.float32) for i in range(M)], axis=0
    )
    return out, res


def kernel(Wh, a):
    return _run(Wh, a)[0]


# revision 14
# speedup vs baseline: 1.2644x; 1.2644x over previous
"""GAT-style attention score kernel for 8 TRN2 NeuronCores, v8.

Computes out[i,j] = LeakyReLU(Wh[i]@a1 + Wh[j]@a2, slope=0.2) for
N=8192, D=64 -> [8192, 8192] f32 output.

Sharding: output rows across 8 cores ([1024, 8192] slab each).

Core idea (HW-verified): ACT applies its per-partition bias BEFORE the
activation table and the Prelu table honors the alpha operand, so with
s2 pre-broadcast (host sends s2b = tile(s2,128) f16) and s1 as an f32
per-partition column, one fused scalar op computes a whole block:

    out[p, f] = Prelu(s2b[p, f] + s1c[p])      # evac+bias+leaky, 1x

Vector covers the rest of each 128-row tile with ts_add (4x, f32
scalar AP) + ts_mul (4x) + tt_max (2x), all f16 SBUF.  No TensorE, no
PSUM.  Steady tiles: S = 2 Prelus on [0:4608] (~4.2us), V = 1 triple
on [4608:8192] (~4.0us).  The last tile shifts work toward V (which
finishes its stream earlier) to shorten the tail.

Output stream: f16 pieces on TWO queues - sync HWDGE carries the
scalar-half pieces, the otherwise-idle gpsimd SWDGE queue carries the
vector-half pieces (all queues share the 16 SDMA engines, which
round-robin at packet granularity).  Host upcasts f16 -> f32.

Startup: the 2MB s2b load is the ramp constraint.  Critical chunks
(s1c, s2b[0:1024], s2b[4608:6400]) ride the scalar HWDGE queue first;
the bulk rides gpsimd gated on s1c landing.  Every input DMA has a
DEDICATED semaphore (a shared counter can hit its threshold via mixed
per-engine completions of different DMAs - this corrupted one core in
two earlier versions).  Tile-0 work is split finer for a fast ramp.
"""

from contextlib import ExitStack

import numpy as np
import concourse.bass as bass
import concourse.mybir as mybir
from concourse.bass_utils import run_bass_kernel_spmd

N = 8192          # nodes
D = 64            # feature dim
M = 8             # cores
ROWS = N // M     # 1024 output rows per core
NT = ROWS // 128  # 8 row tiles of 128 partitions
SW = 4608         # scalar's columns [0:SW] (steady tiles)
SH = SW // 2      # 2304
VMID = SW + (N - SW) // 2  # 6400
TW = 3584         # last-tile scalar/vector split
TH = TW // 2      # 1792
TVM = TW + (N - TW) // 2   # 5888
NEG_SLOPE = 0.2
NOB = 6           # output tile ring depth
XBASE = TW        # x/m buffers cover cols [TW:N]

S0_OPS = [(0, 1024), (1024, SH), (SH, SW)]
V0_OPS = [(SW, VMID), (VMID, N)]

_cache = {}


def _so_val(t, j):
    # S ops: t0 has 3, others 2
    return (j + 1) if t == 0 else 3 + 2 * (t - 1) + j + 1


def _vo_val(t, j=0):
    # V ops: t0 has 2, t1..t6 have 1, t7 has 2
    if t == 0:
        return j + 1
    return 2 + (t - 1) + j + 1


def _build():
    nc = bass.Bass()
    f16 = mybir.dt.float16
    f32 = mybir.dt.float32

    s1c_ext = nc.declare_dram_parameter("s1c", [128, NT], f32, isOutput=False)
    s2b_ext = nc.declare_dram_parameter("s2b", [128, N], f16, isOutput=False)
    out_ext = nc.declare_dram_parameter("out", [ROWS, N], f16, isOutput=True)

    with ExitStack() as ctx:
        sb_s1c = ctx.enter_context(nc.sbuf_tensor("sb_s1c", [128, NT], f32))
        sb_s2b = ctx.enter_context(nc.sbuf_tensor("sb_s2b", [128, N], f16))
        sb_x = ctx.enter_context(nc.sbuf_tensor("sb_x", [128, N - XBASE], f16))
        sb_m = ctx.enter_context(nc.sbuf_tensor("sb_m", [128, N - XBASE], f16))
        sb_o = [
            ctx.enter_context(nc.sbuf_tensor(f"sb_o{i}", [128, N], f16))
            for i in range(NOB)
        ]
        sb_junk = ctx.enter_context(nc.sbuf_tensor("sb_junk", [128, 1], f32))
        dS1 = ctx.enter_context(nc.semaphore("dS1"))      # s1c
        dQ0 = ctx.enter_context(nc.semaphore("dQ0"))      # s2b[0:1024]
        dinB = ctx.enter_context(nc.semaphore("dinB"))    # s2b[4608:6400]
        dinA1 = ctx.enter_context(nc.semaphore("dinA1"))  # s2b[1024:2304]
        dinA2 = ctx.enter_context(nc.semaphore("dinA2"))  # s2b[2304:4608]
        dinC = ctx.enter_context(nc.semaphore("dinC"))    # s2b[6400:8192]
        so = ctx.enter_context(nc.semaphore("so"))
        vo = ctx.enter_context(nc.semaphore("vo"))
        dt = [ctx.enter_context(nc.semaphore(f"dt{t}")) for t in range(NT)]
        block = ctx.enter_context(nc.Block())

        def dtt(t):
            if t == 0 or t == NT - 1:
                return 80 if t == 0 else 64
            return 48

        @block.sync
        def _(sync):
            # all output pieces, emitted in expected readiness order
            # (the queue is FIFO; a stalled head blocks later pieces)
            for t in range(NT):
                ob = sb_o[t % NOB]
                dst = out_ext[t * 128:(t + 1) * 128, :]
                if t == 0:
                    pieces = [
                        (0, 1024, so, _so_val(0, 0)),
                        (1024, SH, so, _so_val(0, 1)),
                        (SW, VMID, vo, _vo_val(0, 0)),
                        (SH, SW, so, _so_val(0, 2)),
                        (VMID, N, vo, _vo_val(0, 1)),
                    ]
                elif t == NT - 1:
                    pieces = [
                        (TW, TVM, vo, _vo_val(t, 0)),
                        (0, TH, so, _so_val(t, 0)),
                        (TH, TW, so, _so_val(t, 1)),
                        (TVM, N, vo, _vo_val(t, 1)),
                    ]
                else:
                    pieces = [
                        (0, SH, so, _so_val(t, 0)),
                        (SH, SW, so, _so_val(t, 1)),
                        (SW, N, vo, _vo_val(t)),
                    ]
                for lo, hi, sem, val in pieces:
                    sync.wait_ge(sem, val)
                    sync.dma_start(dst[:, lo:hi], ob[:, lo:hi]).then_inc(dt[t], 16)

        @block.gpsimd
        def _(gpsimd):
            # bulk s2b load, deferred behind s1c so it can't starve the
            # scalar queue's critical chunks
            gpsimd.wait_ge(dS1, 16)
            gpsimd.dma_start(
                sb_s2b[:, 1024:SH], s2b_ext[:, 1024:SH]
            ).then_inc(dinA1, 16)
            gpsimd.dma_start(
                sb_s2b[:, SH:SW], s2b_ext[:, SH:SW]
            ).then_inc(dinA2, 16)
            gpsimd.dma_start(
                sb_s2b[:, VMID:N], s2b_ext[:, VMID:N]
            ).then_inc(dinC, 16)

        @block.scalar
        def _(scalar):
            scalar.dma_start(sb_s1c[:, :], s1c_ext[:, :]).then_inc(dS1, 16)
            scalar.dma_start(
                sb_s2b[:, 0:1024], s2b_ext[:, 0:1024]
            ).then_inc(dQ0, 16)
            scalar.dma_start(
                sb_s2b[:, SW:VMID], s2b_ext[:, SW:VMID]
            ).then_inc(dinB, 16)
            # warm the Prelu table while they fly
            scalar.activation(
                sb_junk[:, :], sb_junk[:, :],
                mybir.ActivationFunctionType.Prelu,
                bias=0.0, scale=1.0, alpha=NEG_SLOPE,
            )
            for t in range(NT):
                ob = sb_o[t % NOB]
                b = sb_s1c[:, t:t + 1]
                if t >= NOB:
                    scalar.wait_ge(dt[t - NOB], dtt(t - NOB))
                if t == 0:
                    ops = S0_OPS
                elif t == NT - 1:
                    ops = [(0, TH), (TH, TW)]
                else:
                    ops = [(0, SH), (SH, SW)]
                for j, (lo, hi) in enumerate(ops):
                    if t == 0:
                        if j == 0:
                            scalar.wait_ge(dS1, 16)
                            scalar.wait_ge(dQ0, 16)
                        elif j == 1:
                            scalar.wait_ge(dinA1, 16)
                        else:
                            scalar.wait_ge(dinA2, 16)
                    scalar.activation(
                        ob[:, lo:hi], sb_s2b[:, lo:hi],
                        mybir.ActivationFunctionType.Prelu,
                        bias=b, scale=1.0, alpha=NEG_SLOPE,
                    ).then_inc(so)

        @block.vector
        def _(vector):
            for t in range(NT):
                ob = sb_o[t % NOB]
                b = sb_s1c[:, t:t + 1]
                if t >= NOB:
                    vector.wait_ge(dt[t - NOB], dtt(t - NOB))
                if t == 0:
                    ops = V0_OPS
                elif t == NT - 1:
                    ops = [(TW, TVM), (TVM, N)]
                else:
                    ops = [(SW, N)]
                for j, (lo, hi) in enumerate(ops):
                    if t == 0:
                        if j == 0:
                            vector.wait_ge(dS1, 16)
                            vector.wait_ge(dinB, 16)
                        else:
                            vector.wait_ge(dinC, 16)
                    x = sb_x[:, lo - XBASE:hi - XBASE]
                    m = sb_m[:, lo - XBASE:hi - XBASE]
                    vector.tensor_scalar_add(x, sb_s2b[:, lo:hi], b)
                    vector.tensor_scalar_mul(m, x, NEG_SLOPE)
                    vector.tensor_max(ob[:, lo:hi], x, m).then_inc(vo)

    return nc


def _run(Wh, a, trace=False, **kw):
    Wh = np.ascontiguousarray(np.asarray(Wh, dtype=np.float32))
    a = np.ascontiguousarray(np.asarray(a, dtype=np.float32))
    assert Wh.shape == (N, D) and a.shape == (2 * D, 1)

    if "nc" not in _cache:
        _cache["nc"] = _build()
    nc = _cache["nc"]

    s1 = Wh @ a[:D, 0]                         # [N] f32 row contribution
    s2b = np.ascontiguousarray(
        np.broadcast_to((Wh @ a[D:, 0]).astype(np.float16), (128, N))
    )

    in_maps = []
    for i in range(M):
        sl = s1[i * ROWS:(i + 1) * ROWS]
        s1c = np.ascontiguousarray(sl.reshape(NT, 128).T.astype(np.float32))
        in_maps.append({"s1c": s1c, "s2b": s2b})

    res = run_bass_kernel_spmd(nc, in_maps, core_ids=list(range(M)), trace=trace, **kw)
    out = np.concatenate(
        [res.results[i]["out"].astype(np.float32) for i in range(M)], axis=0
    )
    return out, res


def kernel(Wh, a):
    return _run(Wh, a)[0]
